# revision 7
# baseline (speedup 1.0000x reference)
"""Cross-attention Trainium2 kernel (Bass/Tile), data-parallel over batch. v3.

B=8 batch elements -> 8 NeuronCores, one batch element per core.
Per core: y = softmax(q Wq (kv Wk)^T / sqrt(dk)) (kv Wv) Wo + bo
with S1=S2=2048, D=1024, H=8, DK=DV=128.

v3 design (520us v1 -> 478us v2 -> this):
  - host pre-transposes q/kv to [D, S], so all loads are plain strided
    DMAs (no DMA-xbar transposes).
  - fp8 zone = K and Q PROJECTIONS via e4m3 DoubleRow (contraction 256
    per pass, probe-measured at the same 216ns/MM as bf16 at N=512 =
    true 2x).  Host supplies kvT/qT/Wk/Wq in fp8.  This buys the same
    ~55us of PE time as fp8-PV would, but keeps PT in bf16, so the DVE
    softmax tree runs at 2 elem/cycle (fp8 inputs on DVE are 1/cycle,
    which made DVE the phase-2 pacer in v2).  Numerics simulated on the
    real inputs: 1.75e-2 rel err vs the 2e-2 gate; everything else bf16.
  - phase 2 is ACT-paced (8 exps x ~1.11us per head-iter vs ~7.1us of
    attention PE work), so Q-projection (j1..j3) and the output
    projection run as PE filler inside the head-iter g-loop.
  - finalize: DVE tree + ones-matmul broadcast + fast reciprocal, then
    one tensor_mul(OT, ops_psum, rec) writes the normalized OT row
    directly (no separate PSUM->SBUF copy).
  - PSUM (8 banks): scores 2x[128,2,512], PV-accum 1, shared 2-buf pool
    for outproj-y groups + rowsum, qproj filler 1.
"""

import os

import numpy as np

import concourse.bass as bass
import concourse.mybir as mybir
import concourse.tile as tile
from concourse import bacc
from concourse.bass_utils import run_bass_kernel_spmd

B = 8
S = 2048  # S1 == S2
D = 1024  # D1 == D2
H = 8
DK = DV = 128
KC = D // 128  # contraction chunks
SC = S // 128  # sequence chunks of 128
BLK = 512
NBLK = S // BLK
SCALE = 1.0 / float(np.sqrt(DK))
W_WARM = 48

F32 = mybir.dt.float32
BF16 = mybir.dt.bfloat16
F8 = mybir.dt.float8e4
EXP = mybir.ActivationFunctionType.Exp
DR = mybir.MatmulPerfMode.DoubleRow


def _emit(tc, aps):
    nc = tc.nc
    qT8, kvT8, kvT, Wq8, Wk8, Wv, Wo, bo, out = (
        aps["qT8"], aps["kvT8"], aps["kvT"], aps["Wq8"], aps["Wk8"],
        aps["Wv"], aps["Wo"], aps["bo"], aps["out"],
    )
    kvT_v = kvT.rearrange("(kc p) s -> p kc s", p=128)
    kvT8_v = kvT8.rearrange("(kc p) s -> p kc s", p=128)
    qT8_v = qT8.rearrange("(kc p) s -> p kc s", p=128)

    persist = tc.alloc_tile_pool(name="persist", bufs=1)
    QTp = persist.tile([128, H, 2, BLK], BF16, name="QTp")
    KT_sb = persist.tile([128, H, S], BF16, name="KT_sb")
    V_sb = persist.tile([128, SC, H * DV], BF16, name="V_sb")
    Wo_sb = persist.tile([128, KC, D], BF16, name="Wo_sb")
    Wq_sb = persist.tile([128, KC, D], F8, name="Wq_sb")
    bo_bc = persist.tile([128, D], F32, name="bo_bc")
    bo16 = persist.tile([128, D], BF16, name="bo16")
    ones_sb = persist.tile([128, 128], BF16, name="ones_sb")
    oinv_sb = persist.tile([128, 128], BF16, name="oinv_sb")

    nc.vector.memset(ones_sb, 1.0)
    nc.vector.memset(oinv_sb, 1.0 / 128.0)

    def load_weight8(dst, src):
        srcv = src.rearrange("(kc p) n -> p kc n", p=128)
        nc.sync.dma_start(out=dst, in_=srcv)

    def load_weight(dst, src, split=2):
        srcv = src.rearrange("(kc p) n -> p kc n", p=128)
        step = KC // split
        for s in range(split):
            sl = slice(s * step, (s + 1) * step)
            nc.sync.dma_start(out=dst[:, sl, :], in_=srcv[:, sl, :])

    # qT8 stream pool lives through phase 2 (filler consumes j1..j3)
    qstream = tc.alloc_tile_pool(name="qstream", bufs=1)
    # kv8/Wk8 live into early phase 2: K-proj runs m-major as iteration
    # prefixes during block-0 head-iters
    kpool = tc.alloc_tile_pool(name="kpool", bufs=1)
    Wk_sb = kpool.tile([128, KC, D], F8, name="Wk_sb")
    kv8_blocks = {}

    def load_kvT8(j):
        t = kpool.tile([128, KC, BLK], F8, name="kv8b", tag="kv8b", bufs=4)
        nc.sync.dma_start(out=t, in_=kvT8_v[:, :, j * BLK:(j + 1) * BLK])
        return t

    def load_qT8(j):
        t = qstream.tile([128, KC, BLK], F8, name="qTb", tag="qTb", bufs=2)
        nc.sync.dma_start(out=t, in_=qT8_v[:, :, j * BLK:(j + 1) * BLK])
        return t

    # ---- phase 1: K/V projections (+ Q proj of block 0) ---------------
    with nc.named_scope("ph1"), \
         tc.tile_pool(name="p1w", bufs=1) as wpool, \
         tc.tile_pool(name="p1work", bufs=1) as work, \
         tc.tile_pool(name="p1psum", bufs=4, space="PSUM") as pps, \
         tc.tile_pool(name="warmp", bufs=1, space="PSUM") as warmp:
        Wv_sb = wpool.tile([128, KC, D], BF16, name="Wv_sb")

        # warmup chain: keeps the PE issuing while the first DMAs land.
        wps = warmp.tile([128, 128], F32, name="wps")
        for w in range(W_WARM):
            nc.tensor.matmul(
                wps, lhsT=ones_sb, rhs=ones_sb,
                start=(w == 0), stop=(w == W_WARM - 1),
            )

        def load_kvT(j, split=1):
            t = work.tile([128, KC, BLK], BF16, name="kvTb", tag="kvTb",
                          bufs=3)
            step = KC // split
            for s in range(split):
                sl = slice(s * step, (s + 1) * step)
                nc.sync.dma_start(
                    out=t[:, sl, :],
                    in_=kvT_v[:, sl, j * BLK:(j + 1) * BLK],
                )
            return t

        # lead-in: V-proj leads; Wv halves interleave with kvT j0 so
        # V-proj(j0) isn't DMA-starved.  fp8 K/Q inputs load afterwards.
        wv_v = Wv.rearrange("(kc p) n -> p kc n", p=128)
        half = KC // 2
        nc.sync.dma_start(out=Wv_sb[:, 0:half, :], in_=wv_v[:, 0:half, :])
        kv_blocks = {0: load_kvT(0, split=2)}
        nc.sync.dma_start(out=Wv_sb[:, half:, :], in_=wv_v[:, half:, :])
        kv_blocks[1] = load_kvT(1)

        def vproj(x, j):
            for m4 in range(4):
                for n in range(2):
                    ps = pps.tile([128, BLK], F32, name="ps_v", tag="pps")
                    for kc in range(KC):
                        nc.tensor.matmul(
                            ps, lhsT=x[:, kc, m4 * 128:(m4 + 1) * 128],
                            rhs=Wv_sb[:, kc, n * BLK:(n + 1) * BLK],
                            start=(kc == 0), stop=(kc == KC - 1),
                        )
                    nc.scalar.copy(
                        V_sb[:, j * 4 + m4, n * BLK:(n + 1) * BLK], ps
                    )

        for j in range(NBLK):
            x = kv_blocks.pop(j)
            if j + 2 < NBLK:
                kv_blocks[j + 2] = load_kvT(j + 2)
            elif j == 2:
                load_weight8(Wq_sb, Wq8)
            elif j == 3:
                load_weight(Wo_sb, Wo)
            if j == 1:
                # fp8 K/Q inputs: enqueued after the kvT blocks V-proj
                # needs soon; only consumed from the end of phase 1 on
                for jj in range(4):
                    kv8_blocks[jj] = load_kvT8(jj)
                nc.sync.dma_start(
                    out=Wk_sb, in_=Wk8.rearrange("(kc p) n -> p kc n", p=128))
            vproj(x, j)

        # Q-proj block 0 (the rest runs as phase-2 filler)
        qT0 = load_qT8(0)
        qT_blocks = {0: qT0, 1: load_qT8(1)}
        bo_bcast = bass.AP(
            tensor=bo.tensor, offset=bo.offset, ap=[[0, 128]] + list(bo.ap[1:])
        )
        nc.sync.dma_start(out=bo_bc, in_=bo_bcast)
        nc.vector.tensor_copy(bo16, bo_bc)

        def kgroup_ph1(m, j):
            ps = pps.tile([128, BLK], F32, name="ps_k", tag="pps")
            for g in range(KC // 2):
                nc.tensor.matmul(
                    ps,
                    lhsT=Wk_sb[:, 2 * g:2 * g + 2, m * 128:(m + 1) * 128],
                    rhs=kv8_blocks[j][:, 2 * g:2 * g + 2, :],
                    start=(g == 0), stop=(g == KC // 2 - 1),
                    perf_mode=DR,
                )
            nc.vector.tensor_copy(KT_sb[:, m, j * BLK:(j + 1) * BLK], ps)

        def qgroup_ph1(j, m):
            ps = pps.tile([128, BLK], F32, name="ps_q", tag="pps")
            for g in range(KC // 2):
                nc.tensor.matmul(
                    ps,
                    lhsT=Wq_sb[:, 2 * g:2 * g + 2, m * 128:(m + 1) * 128],
                    rhs=qT_blocks[j][:, 2 * g:2 * g + 2, :],
                    start=(g == 0), stop=(g == KC // 2 - 1),
                    perf_mode=DR,
                )
            nc.vector.tensor_copy(QTp[:, m, j % 2, :], ps)

        for j in range(4):
            kgroup_ph1(0, j)
        qgroup_ph1(0, 0)

    # ---- phase 2: attention with Q-proj/outproj filler ----------------
    with nc.named_scope("attn"), \
         tc.tile_pool(name="p2", bufs=1) as p2, \
         tc.tile_pool(name="small", bufs=1) as small, \
         tc.tile_pool(name="spsum", bufs=2, space="PSUM") as spsum, \
         tc.tile_pool(name="opsum", bufs=1, space="PSUM") as opsum, \
         tc.tile_pool(name="mpsum", bufs=2, space="PSUM") as mpsum, \
         tc.tile_pool(name="qpsum", bufs=1, space="PSUM") as qpsum:

        OT_tiles = {}

        class Filler:
            """Emits PE matmuls to fill ACT-paced slack in head-iters."""

            def __init__(self):
                self.groups = []
                self.open = None

            def add_outproj(self, j):
                for m in range(4):
                    for n in range(2):
                        self.groups.append(("o", (j, m, n)))

            def add_qproj(self, j):
                for m in range(H):
                    self.groups.append(("q", (j, m)))

            def _open_next(self):
                if not self.groups:
                    return False
                kind, spec = self.groups.pop(0)
                if kind == "o":
                    ps = mpsum.tile([128, BLK], F32, name="mps", tag="mps")
                else:
                    ps = qpsum.tile([128, BLK], F32, name="qps", tag="qps")
                self.open = (kind, spec, ps, 0)
                return True

            def _emit_one(self):
                kind, spec, ps, i = self.open
                if kind == "o":
                    j, m, n = spec
                    OT = OT_tiles[j]
                    nc.tensor.matmul(
                        ps, lhsT=OT[:, i, m * 128:(m + 1) * 128],
                        rhs=Wo_sb[:, i, n * BLK:(n + 1) * BLK],
                        start=(i == 0), stop=(i == H - 1),
                        skip_group_check=True,
                    )
                    ngroup = 8
                else:
                    j, m = spec
                    qTb = qT_blocks[j]
                    nc.tensor.matmul(
                        ps,
                        lhsT=Wq_sb[:, 2 * i:2 * i + 2, m * 128:(m + 1) * 128],
                        rhs=qTb[:, 2 * i:2 * i + 2, :],
                        start=(i == 0), stop=(i == KC // 2 - 1),
                        perf_mode=DR, skip_group_check=True,
                    )
                    ngroup = KC // 2
                i += 1
                if i == ngroup:
                    self._close(kind, spec, ps)
                    self.open = None
                else:
                    self.open = (kind, spec, ps, i)

            def _close(self, kind, spec, ps):
                if kind == "o":
                    j, m, n = spec
                    y_sb = p2.tile([128, BLK], F32, name="y_sb", tag="y",
                                   bufs=2)
                    nc.vector.tensor_add(
                        y_sb, ps, bo_bc[:, n * BLK:(n + 1) * BLK]
                    )
                    r0 = j * BLK + m * 128
                    nc.sync.dma_start(
                        out=out[r0:r0 + 128, n * BLK:(n + 1) * BLK], in_=y_sb
                    )
                else:
                    j, m = spec
                    nc.vector.tensor_copy(QTp[:, m, j % 2, :], ps)
                    if m == H - 1:
                        del qT_blocks[j]
                        if j + 2 <= NBLK - 1 and j + 2 not in qT_blocks:
                            qT_blocks[j + 2] = load_qT8(j + 2)

            def emit(self, n_mms):
                for _ in range(n_mms):
                    if self.open is None and not self._open_next():
                        return
                    self._emit_one()

            def drain(self):
                while self.open is not None or self.groups:
                    self.emit(1)

        filler = Filler()

        def finalize(j, h, PT, ops):
            """Rowsum via DVE bf16 tree + ones-matmul broadcast + fast
            reciprocal; normalized OT row written straight from PSUM."""
            t8 = small.tile([128, 8, BLK], BF16, name="t8", tag="t8", bufs=1)
            nc.vector.tensor_add(t8, PT[:, 0:8, :], PT[:, 8:16, :])
            nc.vector.tensor_add(t8[:, 0:4], t8[:, 0:4], t8[:, 4:8])
            nc.vector.tensor_add(t8[:, 0:2], t8[:, 0:2], t8[:, 2:4])
            nc.vector.tensor_add(t8[:, 0, :], t8[:, 0, :], t8[:, 1, :])
            rps = mpsum.tile([128, BLK], F32, name="mps", tag="mps")
            nc.tensor.matmul(
                rps, lhsT=ones_sb, rhs=t8[:, 0, :], start=True, stop=True
            )
            rec = small.tile([128, BLK], F32, name="rec", tag="rec", bufs=2)
            nc.vector.reciprocal_approx_fast(out=rec, in_=rps)
            nc.vector.tensor_mul(OT_tiles[j][:, h, :], ops, rec)

        def pgroup(kind, m, j, pool):
            # phase-2 K/Q projection prefix group (m-major pipeline)
            if pool == 0:
                ps = qpsum.tile([128, BLK], F32, name="qps", tag="qps")
            else:
                ps = mpsum.tile([128, BLK], F32, name="mps", tag="mps")
            W = Wk_sb if kind == "k" else Wq_sb
            x = kv8_blocks[j] if kind == "k" else qT_blocks[j]
            for g in range(KC // 2):
                nc.tensor.matmul(
                    ps, lhsT=W[:, 2 * g:2 * g + 2, m * 128:(m + 1) * 128],
                    rhs=x[:, 2 * g:2 * g + 2, :],
                    start=(g == 0), stop=(g == KC // 2 - 1),
                    perf_mode=DR, skip_group_check=True,
                )
            if kind == "k":
                nc.vector.tensor_copy(KT_sb[:, m, j * BLK:(j + 1) * BLK], ps)
            else:
                nc.vector.tensor_copy(QTp[:, m, j % 2, :], ps)

        def prefix(h):
            # during iter (0,h): project K(m=h+1) all blocks, Q(j0,m=h+1),
            # Q(j1,m=h) -- each consumed exactly one iteration later
            pool = 0
            if h < H - 1:
                for jj in range(4):
                    pgroup("k", h + 1, jj, pool)
                    pool ^= 1
                pgroup("q", h + 1, 0, pool)
                pool ^= 1
            pgroup("q", h, 1, pool)
            if h == H - 2:
                del qT_blocks[0]
                qT_blocks[2] = load_qT8(2)
            elif h == H - 1:
                del qT_blocks[1]
                qT_blocks[3] = load_qT8(3)

        def head_iter(cur, prev):
            j, h = cur
            if j == 0:
                prefix(h)
            PT = p2.tile([128, SC, BLK], BF16, name="PT", tag="PT", bufs=2)
            qblk = QTp[:, h, j % 2, :]
            if prev is not None:
                pj, ph, pPT = prev
                ops = opsum.tile([128, BLK], F32, name="ops", tag="ops")
            for g in range(SC // 2):
                sps = spsum.tile([128, 2, BLK], F32, name="sps", tag="sps")
                for i in range(2):
                    c = 2 * g + i
                    nc.tensor.matmul(
                        sps[:, i, :],
                        lhsT=KT_sb[:, h, c * 128:(c + 1) * 128],
                        rhs=qblk, start=True, stop=True,
                    )
                if prev is not None:
                    for i in range(2):
                        c = 2 * g + i
                        nc.tensor.matmul(
                            ops, lhsT=V_sb[:, c, ph * 128:(ph + 1) * 128],
                            rhs=pPT[:, c, :], start=(c == 0), stop=(c == SC - 1),
                            skip_group_check=True,
                        )
                filler.emit(2)
                nc.scalar.activation(
                    PT[:, 2 * g:2 * (g + 1), :], sps, EXP, scale=SCALE
                )
            if prev is not None:
                finalize(pj, ph, pPT, ops)
            return PT

        seq = [(j, h) for j in range(NBLK) for h in range(H)]
        prev = None
        for j, h in seq:
            if h == 0:
                OT_tiles[j] = p2.tile(
                    [128, H, BLK], BF16, name="OT", tag="OT", bufs=2
                )
                if 1 <= j < NBLK - 1:
                    filler.add_qproj(j + 1)
            if h == 1 and j > 0:
                filler.add_outproj(j - 1)
            PT = head_iter((j, h), prev)
            prev = (j, h, PT)

        # ---- tail: PV of the last head + outproj(3) ------------------
        lj, lh, lPT = prev
        filler.drain()
        OT = OT_tiles[lj]

        def ygroup_partial(m, n, pool):
            # bias is preloaded via (ones/128).T @ bo16 so the close needs
            # only a plain PSUM->SBUF copy (ACT or DVE, both idle in tail)
            if pool == 0:
                yps = mpsum.tile([128, BLK], F32, name="mps", tag="mps")
            else:
                yps = qpsum.tile([128, BLK], F32, name="qps", tag="qps")
            nc.tensor.matmul(
                yps, lhsT=oinv_sb, rhs=bo16[:, n * BLK:(n + 1) * BLK],
                start=True, stop=False, skip_group_check=True,
            )
            for hh in range(H - 1):
                nc.tensor.matmul(
                    yps, lhsT=OT[:, hh, m * 128:(m + 1) * 128],
                    rhs=Wo_sb[:, hh, n * BLK:(n + 1) * BLK],
                    start=False, stop=False, skip_group_check=True,
                )
            return yps

        def ygroup_close(m, n, yps, eng):
            nc.tensor.matmul(
                yps, lhsT=OT[:, H - 1, m * 128:(m + 1) * 128],
                rhs=Wo_sb[:, H - 1, n * BLK:(n + 1) * BLK],
                start=False, stop=True, skip_group_check=True,
            )
            y_sb = p2.tile([128, BLK], F32, name="y_sb", tag="y", bufs=2)
            if eng == 0:
                nc.scalar.copy(y_sb, yps)
            else:
                nc.vector.tensor_copy(y_sb, yps)
            r0 = lj * BLK + m * 128
            nc.sync.dma_start(
                out=out[r0:r0 + 128, n * BLK:(n + 1) * BLK], in_=y_sb
            )

        groups = [(m, n) for m in range(4) for n in range(2)]
        openg = [(groups[0], ygroup_partial(*groups[0], pool=0)),
                 (groups[1], ygroup_partial(*groups[1], pool=1))]
        ops = opsum.tile([128, BLK], F32, name="ops", tag="ops")
        for c in range(SC):
            nc.tensor.matmul(
                ops, lhsT=V_sb[:, c, lh * 128:(lh + 1) * 128],
                rhs=lPT[:, c, :], start=(c == 0), stop=(c == SC - 1),
                skip_group_check=True,
            )
        finalize(lj, lh, lPT, ops)
        for t in range(8):
            (m, n), yps = openg.pop(0)
            ygroup_close(m, n, yps, eng=t % 2)
            if t + 2 < 8:
                gnext = groups[t + 2]
                openg.append((gnext, ygroup_partial(*gnext, pool=t % 2)))
    kpool.release()
    qstream.release()
    persist.release()


_CACHE = {}


def _build():
    if "nc" in _CACHE:
        return _CACHE["nc"]
    nc = bacc.Bacc(
        "TRN2", target_bir_lowering=False, debug=False,
        enable_asserts=False, num_devices=B,
    )
    aps = {
        "qT8": nc.dram_tensor("qT8", [D, S], F8, kind="ExternalInput").ap(),
        "kvT8": nc.dram_tensor("kvT8", [D, S], F8, kind="ExternalInput").ap(),
        "kvT": nc.dram_tensor("kvT", [D, S], BF16, kind="ExternalInput").ap(),
        "Wq8": nc.dram_tensor("Wq8", [D, H * DK], F8, kind="ExternalInput").ap(),
        "Wk8": nc.dram_tensor("Wk8", [D, H * DK], F8, kind="ExternalInput").ap(),
        "Wv": nc.dram_tensor("Wv", [D, H * DV], BF16, kind="ExternalInput").ap(),
        "Wo": nc.dram_tensor("Wo", [H * DV, D], BF16, kind="ExternalInput").ap(),
        "bo": nc.dram_tensor("bo", [1, D], F32, kind="ExternalInput").ap(),
        "out": nc.dram_tensor("out", [S, D], F32, kind="ExternalOutput").ap(),
    }
    with tile.TileContext(nc) as tc:
        _emit(tc, aps)
    nc.compile()
    _CACHE["nc"] = nc
    return nc


LAST_RESULT = None


def kernel(query, key_value, Wq, Wk, Wv, Wo, bo):
    global LAST_RESULT
    import ml_dtypes

    BF = ml_dtypes.bfloat16
    E4 = ml_dtypes.float8_e4m3fn
    nc = _build()
    # host-side: pre-transpose inputs to [D, S]; fp8 copies for K/Q proj
    qT = np.asarray(query, dtype=np.float32).transpose(0, 2, 1)
    kvT = np.asarray(key_value, dtype=np.float32).transpose(0, 2, 1)
    qT8 = np.ascontiguousarray(qT.astype(E4))
    kvT8 = np.ascontiguousarray(kvT.astype(E4))
    kvT_b = np.ascontiguousarray(kvT.astype(BF))
    shared = {
        "Wq8": np.ascontiguousarray(np.asarray(Wq, dtype=np.float32).astype(E4)),
        "Wk8": np.ascontiguousarray(np.asarray(Wk, dtype=np.float32).astype(E4)),
        "Wv": np.ascontiguousarray(np.asarray(Wv, dtype=np.float32).astype(BF)),
        "Wo": np.ascontiguousarray(np.asarray(Wo, dtype=np.float32).astype(BF)),
        "bo": np.ascontiguousarray(np.asarray(bo, dtype=np.float32)).reshape(1, D),
    }
    in_maps = [
        {"qT8": qT8[i], "kvT8": kvT8[i], "kvT": kvT_b[i], **shared}
        for i in range(B)
    ]
    res = run_bass_kernel_spmd(
        nc, in_maps, core_ids=list(range(B)),
        trace=bool(int(os.environ.get("KERNEL_TRACE", "0"))),
    )
    LAST_RESULT = res
    return np.stack([r["out"] for r in res.results]).astype(np.float32)


if __name__ == "__main__":
    rng = np.random.default_rng(0)
    inputs = {
        "query": rng.standard_normal((B, S, D), dtype=np.float32),
        "key_value": rng.standard_normal((B, S, D), dtype=np.float32),
        "Wq": (rng.random((D, H * DK), dtype=np.float32) - 0.5) / 16.0,
        "Wk": (rng.random((D, H * DK), dtype=np.float32) - 0.5) / 16.0,
        "Wv": (rng.random((D, H * DV), dtype=np.float32) - 0.5) / 16.0,
        "Wo": (rng.random((H * DV, D), dtype=np.float32) - 0.5) / 16.0,
        "bo": (rng.random(D, dtype=np.float32) - 0.5) / 16.0,
    }
    y = kernel(**inputs)
    print("kernel out", y.shape, y.dtype, float(np.abs(y).max()))


# revision 8
# speedup vs baseline: 1.1910x; 1.1910x over previous
"""Cross-attention Trainium2 kernel (Bass/Tile), data-parallel over batch. v3.

B=8 batch elements -> 8 NeuronCores, one batch element per core.
Per core: y = softmax(q Wq (kv Wk)^T / sqrt(dk)) (kv Wv) Wo + bo
with S1=S2=2048, D=1024, H=8, DK=DV=128.

v3 design (520us v1 -> 478us v2 -> this):
  - host pre-transposes q/kv to [D, S], so all loads are plain strided
    DMAs (no DMA-xbar transposes).
  - fp8 zone = K and Q PROJECTIONS via e4m3 DoubleRow (contraction 256
    per pass, probe-measured at the same 216ns/MM as bf16 at N=512 =
    true 2x).  Host supplies kvT/qT/Wk/Wq in fp8.  This buys the same
    ~55us of PE time as fp8-PV would, but keeps PT in bf16, so the DVE
    softmax tree runs at 2 elem/cycle (fp8 inputs on DVE are 1/cycle,
    which made DVE the phase-2 pacer in v2).  Numerics simulated on the
    real inputs: 1.75e-2 rel err vs the 2e-2 gate; everything else bf16.
  - phase 2 is ACT-paced (8 exps x ~1.11us per head-iter vs ~7.1us of
    attention PE work), so Q-projection (j1..j3) and the output
    projection run as PE filler inside the head-iter g-loop.
  - finalize: DVE tree + ones-matmul broadcast + fast reciprocal, then
    one tensor_mul(OT, ops_psum, rec) writes the normalized OT row
    directly (no separate PSUM->SBUF copy).
  - PSUM (8 banks): scores 2x[128,2,512], PV-accum 1, shared 2-buf pool
    for outproj-y groups + rowsum, qproj filler 1.
"""

import os

import numpy as np

import concourse.bass as bass
import concourse.mybir as mybir
import concourse.tile as tile
from concourse import bacc
from concourse.bass_utils import run_bass_kernel_spmd

B = 8
S = 2048  # S1 == S2
D = 1024  # D1 == D2
H = 8
DK = DV = 128
KC = D // 128  # contraction chunks
SC = S // 128  # sequence chunks of 128
BLK = 512
NBLK = S // BLK
SCALE = 1.0 / float(np.sqrt(DK))
W_WARM = 48

F32 = mybir.dt.float32
BF16 = mybir.dt.bfloat16
F8 = mybir.dt.float8e4
EXP = mybir.ActivationFunctionType.Exp
DR = mybir.MatmulPerfMode.DoubleRow


def _emit(tc, aps):
    nc = tc.nc
    qT8, kvT8, kvT, Wq8, Wk8, Wv, Wo, bo, out = (
        aps["qT8"], aps["kvT8"], aps["kvT"], aps["Wq8"], aps["Wk8"],
        aps["Wv"], aps["Wo"], aps["bo"], aps["out"],
    )
    kvT_v = kvT.rearrange("(kc p) s -> p kc s", p=128)
    kvT8_v = kvT8.rearrange("(kc p) s -> p kc s", p=128)
    qT8_v = qT8.rearrange("(kc p) s -> p kc s", p=128)

    persist = tc.alloc_tile_pool(name="persist", bufs=1)
    QT_sb = persist.tile([128, H, S], BF16, name="QT_sb")
    KT_sb = persist.tile([128, H, S], BF16, name="KT_sb")
    V_sb = persist.tile([128, SC, H * DV], BF16, name="V_sb")
    Wo_sb = persist.tile([128, KC, D], BF16, name="Wo_sb")
    Wq_sb = persist.tile([128, KC, D], F8, name="Wq_sb")
    bo_bc = persist.tile([128, D], F32, name="bo_bc")
    bo16 = persist.tile([128, D], BF16, name="bo16")
    ones_sb = persist.tile([128, 128], BF16, name="ones_sb")
    oinv_sb = persist.tile([128, 128], BF16, name="oinv_sb")

    nc.vector.memset(ones_sb, 1.0)
    nc.vector.memset(oinv_sb, 1.0 / 128.0)

    def load_weight8(dst, src):
        srcv = src.rearrange("(kc p) n -> p kc n", p=128)
        nc.sync.dma_start(out=dst, in_=srcv)

    def load_weight(dst, src, split=2):
        srcv = src.rearrange("(kc p) n -> p kc n", p=128)
        step = KC // split
        for s in range(split):
            sl = slice(s * step, (s + 1) * step)
            nc.sync.dma_start(out=dst[:, sl, :], in_=srcv[:, sl, :])

    # qT8 stream pool lives through phase 2 (filler consumes j1..j3)
    qstream = tc.alloc_tile_pool(name="qstream", bufs=1)

    def load_qT8(j):
        t = qstream.tile([128, KC, BLK], F8, name="qTb", tag="qTb", bufs=2)
        nc.sync.dma_start(out=t, in_=qT8_v[:, :, j * BLK:(j + 1) * BLK])
        return t

    # ---- phase 1: K/V projections (+ Q proj of block 0) ---------------
    with nc.named_scope("ph1"), \
         tc.tile_pool(name="p1w", bufs=1) as wpool, \
         tc.tile_pool(name="p1work", bufs=1) as work, \
         tc.tile_pool(name="p1psum", bufs=4, space="PSUM") as pps, \
         tc.tile_pool(name="warmp", bufs=1, space="PSUM") as warmp:
        Wk_sb = wpool.tile([128, KC, D], F8, name="Wk_sb")
        Wv_sb = wpool.tile([128, KC, D], BF16, name="Wv_sb")

        # warmup chain: keeps the PE issuing while the first DMAs land.
        wps = warmp.tile([128, 128], F32, name="wps")
        for w in range(W_WARM):
            nc.tensor.matmul(
                wps, lhsT=ones_sb, rhs=ones_sb,
                start=(w == 0), stop=(w == W_WARM - 1),
            )

        def load_kvT8(j):
            t = work.tile([128, KC, BLK], F8, name="kv8b", tag="kv8b", bufs=3)
            nc.sync.dma_start(out=t, in_=kvT8_v[:, :, j * BLK:(j + 1) * BLK])
            return t

        def load_kvT(j, split=1):
            t = work.tile([128, KC, BLK], BF16, name="kvTb", tag="kvTb",
                          bufs=3)
            step = KC // split
            for s in range(split):
                sl = slice(s * step, (s + 1) * step)
                nc.sync.dma_start(
                    out=t[:, sl, :],
                    in_=kvT_v[:, sl, j * BLK:(j + 1) * BLK],
                )
            return t

        # lead-in: fp8 K inputs are small; K-proj starts early.  Wv and
        # kvT j0 are interleaved so V-proj(j0) isn't DMA-starved.
        nc.sync.dma_start(out=Wk_sb, in_=Wk8.rearrange("(kc p) n -> p kc n",
                                                        p=128))
        kv8_blocks = {0: load_kvT8(0)}
        wv_v = Wv.rearrange("(kc p) n -> p kc n", p=128)
        half = KC // 2
        nc.sync.dma_start(out=Wv_sb[:, 0:half, :], in_=wv_v[:, 0:half, :])
        kv_blocks = {0: load_kvT(0)}
        nc.sync.dma_start(out=Wv_sb[:, half:, :], in_=wv_v[:, half:, :])
        kv8_blocks[1] = load_kvT8(1)
        kv_blocks[1] = load_kvT(1)

        def kproj(x8, j):
            # DoubleRow: 4 accumulating MMs over 256-contraction pairs
            for m in range(H):
                ps = pps.tile([128, BLK], F32, name="ps_k", tag="pps")
                for g in range(KC // 2):
                    nc.tensor.matmul(
                        ps,
                        lhsT=Wk_sb[:, 2 * g:2 * g + 2, m * 128:(m + 1) * 128],
                        rhs=x8[:, 2 * g:2 * g + 2, :],
                        start=(g == 0), stop=(g == KC // 2 - 1),
                        perf_mode=DR,
                    )
                nc.vector.tensor_copy(KT_sb[:, m, j * BLK:(j + 1) * BLK], ps)

        def vproj(x, j):
            for m4 in range(4):
                for n in range(2):
                    ps = pps.tile([128, BLK], F32, name="ps_v", tag="pps")
                    for kc in range(KC):
                        nc.tensor.matmul(
                            ps, lhsT=x[:, kc, m4 * 128:(m4 + 1) * 128],
                            rhs=Wv_sb[:, kc, n * BLK:(n + 1) * BLK],
                            start=(kc == 0), stop=(kc == KC - 1),
                        )
                    nc.scalar.copy(
                        V_sb[:, j * 4 + m4, n * BLK:(n + 1) * BLK], ps
                    )

        for j in range(NBLK):
            x8 = kv8_blocks.pop(j)
            x = kv_blocks.pop(j)
            if j + 2 < NBLK:
                kv8_blocks[j + 2] = load_kvT8(j + 2)
                kv_blocks[j + 2] = load_kvT(j + 2)
            elif j == 2:
                load_weight8(Wq_sb, Wq8)
            elif j == 3:
                load_weight(Wo_sb, Wo)
            kproj(x8, j)
            vproj(x, j)

        # Q-proj block 0 (the rest runs as phase-2 filler)
        qT0 = load_qT8(0)
        qT_blocks = {0: qT0, 1: load_qT8(1)}
        bo_bcast = bass.AP(
            tensor=bo.tensor, offset=bo.offset, ap=[[0, 128]] + list(bo.ap[1:])
        )
        nc.sync.dma_start(out=bo_bc, in_=bo_bcast)
        nc.vector.tensor_copy(bo16, bo_bc)
        for m in range(H):
            ps = pps.tile([128, BLK], F32, name="ps_q", tag="pps")
            for g in range(KC // 2):
                nc.tensor.matmul(
                    ps,
                    lhsT=Wq_sb[:, 2 * g:2 * g + 2, m * 128:(m + 1) * 128],
                    rhs=qT0[:, 2 * g:2 * g + 2, :],
                    start=(g == 0), stop=(g == KC // 2 - 1),
                    perf_mode=DR,
                )
            nc.vector.tensor_copy(QT_sb[:, m, 0:BLK], ps)
        del qT_blocks[0]
        qT_blocks[2] = load_qT8(2)

    # ---- phase 2: attention with Q-proj/outproj filler ----------------
    with nc.named_scope("attn"), \
         tc.tile_pool(name="p2", bufs=1) as p2, \
         tc.tile_pool(name="small", bufs=1) as small, \
         tc.tile_pool(name="spsum", bufs=2, space="PSUM") as spsum, \
         tc.tile_pool(name="opsum", bufs=1, space="PSUM") as opsum, \
         tc.tile_pool(name="mpsum", bufs=2, space="PSUM") as mpsum, \
         tc.tile_pool(name="qpsum", bufs=1, space="PSUM") as qpsum:

        OT_tiles = {}

        class Filler:
            """Emits PE matmuls to fill ACT-paced slack in head-iters."""

            def __init__(self):
                self.groups = []
                self.open = None

            def add_outproj(self, j):
                for m in range(4):
                    for n in range(2):
                        self.groups.append(("o", (j, m, n)))

            def add_qproj(self, j):
                for m in range(H):
                    self.groups.append(("q", (j, m)))

            def _open_next(self):
                if not self.groups:
                    return False
                kind, spec = self.groups.pop(0)
                if kind == "o":
                    ps = mpsum.tile([128, BLK], F32, name="mps", tag="mps")
                else:
                    ps = qpsum.tile([128, BLK], F32, name="qps", tag="qps")
                self.open = (kind, spec, ps, 0)
                return True

            def _emit_one(self):
                kind, spec, ps, i = self.open
                if kind == "o":
                    j, m, n = spec
                    OT = OT_tiles[j]
                    nc.tensor.matmul(
                        ps, lhsT=OT[:, i, m * 128:(m + 1) * 128],
                        rhs=Wo_sb[:, i, n * BLK:(n + 1) * BLK],
                        start=(i == 0), stop=(i == H - 1),
                        skip_group_check=True,
                    )
                    ngroup = 8
                else:
                    j, m = spec
                    qTb = qT_blocks[j]
                    nc.tensor.matmul(
                        ps,
                        lhsT=Wq_sb[:, 2 * i:2 * i + 2, m * 128:(m + 1) * 128],
                        rhs=qTb[:, 2 * i:2 * i + 2, :],
                        start=(i == 0), stop=(i == KC // 2 - 1),
                        perf_mode=DR, skip_group_check=True,
                    )
                    ngroup = KC // 2
                i += 1
                if i == ngroup:
                    self._close(kind, spec, ps)
                    self.open = None
                else:
                    self.open = (kind, spec, ps, i)

            def _close(self, kind, spec, ps):
                if kind == "o":
                    j, m, n = spec
                    y_sb = p2.tile([128, BLK], F32, name="y_sb", tag="y",
                                   bufs=2)
                    nc.vector.tensor_add(
                        y_sb, ps, bo_bc[:, n * BLK:(n + 1) * BLK]
                    )
                    r0 = j * BLK + m * 128
                    nc.sync.dma_start(
                        out=out[r0:r0 + 128, n * BLK:(n + 1) * BLK], in_=y_sb
                    )
                else:
                    j, m = spec
                    nc.vector.tensor_copy(
                        QT_sb[:, m, j * BLK:(j + 1) * BLK], ps
                    )
                    if m == H - 1:
                        del qT_blocks[j]
                        if j + 2 <= NBLK - 1 and j + 2 not in qT_blocks:
                            qT_blocks[j + 2] = load_qT8(j + 2)

            def emit(self, n_mms):
                for _ in range(n_mms):
                    if self.open is None and not self._open_next():
                        return
                    self._emit_one()

            def drain(self):
                while self.open is not None or self.groups:
                    self.emit(1)

        filler = Filler()

        def finalize(j, h, PT, ops):
            """Rowsum via DVE bf16 tree + ones-matmul broadcast + fast
            reciprocal; normalized OT row written straight from PSUM."""
            t8 = small.tile([128, 8, BLK], BF16, name="t8", tag="t8", bufs=1)
            nc.vector.tensor_add(t8, PT[:, 0:8, :], PT[:, 8:16, :])
            nc.vector.tensor_add(t8[:, 0:4], t8[:, 0:4], t8[:, 4:8])
            nc.vector.tensor_add(t8[:, 0:2], t8[:, 0:2], t8[:, 2:4])
            nc.vector.tensor_add(t8[:, 0, :], t8[:, 0, :], t8[:, 1, :])
            rps = mpsum.tile([128, BLK], F32, name="mps", tag="mps")
            nc.tensor.matmul(
                rps, lhsT=ones_sb, rhs=t8[:, 0, :], start=True, stop=True
            )
            rec = small.tile([128, BLK], F32, name="rec", tag="rec", bufs=2)
            nc.vector.reciprocal_approx_fast(out=rec, in_=rps)
            nc.vector.tensor_mul(OT_tiles[j][:, h, :], ops, rec)

        def head_iter(cur, prev):
            j, h = cur
            PT = p2.tile([128, SC, BLK], BF16, name="PT", tag="PT", bufs=2)
            qblk = QT_sb[:, h, j * BLK:(j + 1) * BLK]
            if prev is not None:
                pj, ph, pPT = prev
                ops = opsum.tile([128, BLK], F32, name="ops", tag="ops")
            for g in range(SC // 2):
                sps = spsum.tile([128, 2, BLK], F32, name="sps", tag="sps")
                for i in range(2):
                    c = 2 * g + i
                    nc.tensor.matmul(
                        sps[:, i, :],
                        lhsT=KT_sb[:, h, c * 128:(c + 1) * 128],
                        rhs=qblk, start=True, stop=True,
                    )
                if prev is not None:
                    for i in range(2):
                        c = 2 * g + i
                        nc.tensor.matmul(
                            ops, lhsT=V_sb[:, c, ph * 128:(ph + 1) * 128],
                            rhs=pPT[:, c, :], start=(c == 0), stop=(c == SC - 1),
                            skip_group_check=True,
                        )
                filler.emit(2)
                nc.scalar.activation(
                    PT[:, 2 * g:2 * (g + 1), :], sps, EXP, scale=SCALE
                )
            if prev is not None:
                finalize(pj, ph, pPT, ops)
            return PT

        seq = [(j, h) for j in range(NBLK) for h in range(H)]
        prev = None
        for j, h in seq:
            if h == 0:
                OT_tiles[j] = p2.tile(
                    [128, H, BLK], BF16, name="OT", tag="OT", bufs=2
                )
                if j + 1 < NBLK:
                    filler.add_qproj(j + 1)
            if h == 1 and j > 0:
                filler.add_outproj(j - 1)
            PT = head_iter((j, h), prev)
            prev = (j, h, PT)

        # ---- tail: PV of the last head + outproj(3) ------------------
        lj, lh, lPT = prev
        filler.drain()
        OT = OT_tiles[lj]

        def ygroup_partial(m, n, pool):
            # bias is preloaded via (ones/128).T @ bo16 so the close needs
            # only a plain PSUM->SBUF copy (ACT or DVE, both idle in tail)
            if pool == 0:
                yps = mpsum.tile([128, BLK], F32, name="mps", tag="mps")
            else:
                yps = qpsum.tile([128, BLK], F32, name="qps", tag="qps")
            nc.tensor.matmul(
                yps, lhsT=oinv_sb, rhs=bo16[:, n * BLK:(n + 1) * BLK],
                start=True, stop=False, skip_group_check=True,
            )
            for hh in range(H - 1):
                nc.tensor.matmul(
                    yps, lhsT=OT[:, hh, m * 128:(m + 1) * 128],
                    rhs=Wo_sb[:, hh, n * BLK:(n + 1) * BLK],
                    start=False, stop=False, skip_group_check=True,
                )
            return yps

        def ygroup_close(m, n, yps, eng):
            nc.tensor.matmul(
                yps, lhsT=OT[:, H - 1, m * 128:(m + 1) * 128],
                rhs=Wo_sb[:, H - 1, n * BLK:(n + 1) * BLK],
                start=False, stop=True, skip_group_check=True,
            )
            y_sb = p2.tile([128, BLK], F32, name="y_sb", tag="y", bufs=2)
            if eng == 0:
                nc.scalar.copy(y_sb, yps)
            else:
                nc.vector.tensor_copy(y_sb, yps)
            r0 = lj * BLK + m * 128
            nc.sync.dma_start(
                out=out[r0:r0 + 128, n * BLK:(n + 1) * BLK], in_=y_sb
            )

        groups = [(m, n) for m in range(4) for n in range(2)]
        openg = [(groups[0], ygroup_partial(*groups[0], pool=0)),
                 (groups[1], ygroup_partial(*groups[1], pool=1))]
        ops = opsum.tile([128, BLK], F32, name="ops", tag="ops")
        for c in range(SC):
            nc.tensor.matmul(
                ops, lhsT=V_sb[:, c, lh * 128:(lh + 1) * 128],
                rhs=lPT[:, c, :], start=(c == 0), stop=(c == SC - 1),
                skip_group_check=True,
            )
        finalize(lj, lh, lPT, ops)
        for t in range(8):
            (m, n), yps = openg.pop(0)
            ygroup_close(m, n, yps, eng=t % 2)
            if t + 2 < 8:
                gnext = groups[t + 2]
                openg.append((gnext, ygroup_partial(*gnext, pool=t % 2)))
    qstream.release()
    persist.release()


_CACHE = {}


def _build():
    if "nc" in _CACHE:
        return _CACHE["nc"]
    nc = bacc.Bacc(
        "TRN2", target_bir_lowering=False, debug=False,
        enable_asserts=False, num_devices=B,
    )
    aps = {
        "qT8": nc.dram_tensor("qT8", [D, S], F8, kind="ExternalInput").ap(),
        "kvT8": nc.dram_tensor("kvT8", [D, S], F8, kind="ExternalInput").ap(),
        "kvT": nc.dram_tensor("kvT", [D, S], BF16, kind="ExternalInput").ap(),
        "Wq8": nc.dram_tensor("Wq8", [D, H * DK], F8, kind="ExternalInput").ap(),
        "Wk8": nc.dram_tensor("Wk8", [D, H * DK], F8, kind="ExternalInput").ap(),
        "Wv": nc.dram_tensor("Wv", [D, H * DV], BF16, kind="ExternalInput").ap(),
        "Wo": nc.dram_tensor("Wo", [H * DV, D], BF16, kind="ExternalInput").ap(),
        "bo": nc.dram_tensor("bo", [1, D], F32, kind="ExternalInput").ap(),
        "out": nc.dram_tensor("out", [S, D], F32, kind="ExternalOutput").ap(),
    }
    with tile.TileContext(nc) as tc:
        _emit(tc, aps)
    nc.compile()
    _CACHE["nc"] = nc
    return nc


LAST_RESULT = None


def kernel(query, key_value, Wq, Wk, Wv, Wo, bo):
    global LAST_RESULT
    import ml_dtypes

    BF = ml_dtypes.bfloat16
    E4 = ml_dtypes.float8_e4m3fn
    nc = _build()
    # host-side: pre-transpose inputs to [D, S]; fp8 copies for K/Q proj
    qT = np.asarray(query, dtype=np.float32).transpose(0, 2, 1)
    kvT = np.asarray(key_value, dtype=np.float32).transpose(0, 2, 1)
    qT8 = np.ascontiguousarray(qT.astype(E4))
    kvT8 = np.ascontiguousarray(kvT.astype(E4))
    kvT_b = np.ascontiguousarray(kvT.astype(BF))
    shared = {
        "Wq8": np.ascontiguousarray(np.asarray(Wq, dtype=np.float32).astype(E4)),
        "Wk8": np.ascontiguousarray(np.asarray(Wk, dtype=np.float32).astype(E4)),
        "Wv": np.ascontiguousarray(np.asarray(Wv, dtype=np.float32).astype(BF)),
        "Wo": np.ascontiguousarray(np.asarray(Wo, dtype=np.float32).astype(BF)),
        "bo": np.ascontiguousarray(np.asarray(bo, dtype=np.float32)).reshape(1, D),
    }
    in_maps = [
        {"qT8": qT8[i], "kvT8": kvT8[i], "kvT": kvT_b[i], **shared}
        for i in range(B)
    ]
    res = run_bass_kernel_spmd(
        nc, in_maps, core_ids=list(range(B)),
        trace=bool(int(os.environ.get("KERNEL_TRACE", "0"))),
    )
    LAST_RESULT = res
    return np.stack([r["out"] for r in res.results]).astype(np.float32)


if __name__ == "__main__":
    rng = np.random.default_rng(0)
    inputs = {
        "query": rng.standard_normal((B, S, D), dtype=np.float32),
        "key_value": rng.standard_normal((B, S, D), dtype=np.float32),
        "Wq": (rng.random((D, H * DK), dtype=np.float32) - 0.5) / 16.0,
        "Wk": (rng.random((D, H * DK), dtype=np.float32) - 0.5) / 16.0,
        "Wv": (rng.random((D, H * DV), dtype=np.float32) - 0.5) / 16.0,
        "Wo": (rng.random((H * DV, D), dtype=np.float32) - 0.5) / 16.0,
        "bo": (rng.random(D, dtype=np.float32) - 0.5) / 16.0,
    }
    y = kernel(**inputs)
    print("kernel out", y.shape, y.dtype, float(np.abs(y).max()))
